# revision 3
# baseline (speedup 1.0000x reference)
"""Trainium2 Bass kernel for the BAHDANAU+ group-recommendation model.

kernel(**inputs) takes the complete (unsharded) numpy inputs, shards the
131072-query batch across 8 NeuronCores (data-parallel, tables replicated),
runs the Bass kernel SPMD, and returns the full [B, 1] float32 output.

Design:
  - Host-side table join (query-independent prep): trip_tab[g] =
    user_emb[members[g]] as a [500K, 96] bf16 table, and item_tab =
    item_emb || genres as [100K, 32] bf16.  One 192B indirect gather
    (trip) + one 64B gather (item) per 128-row tile = 2 SWDGE DMAs per
    tile (256 per core), round-robin across the 4 SWDGE queues.
  - Compute in a transposed layout per 4-tile sub-batch (n=512 columns),
    feature order [item, mem0, mem1, mem2] so every matmul operand starts
    at SBUF partition 0:
      PE: per-tile 128x128 bf16 transpose (gi -> giT, via identity);
      PE: at = WE^T giT (attention logits pre-broadcast to the 96 member
          rows; rows 0:31 zero);
      DVE: mw = (at + b') * giT  (fused bias + elementwise product);
      PE: g = S^T mw (block-sum of the 3 weighted member embeddings);
      DVE: mw[0:32] = g * itT  (the g*item product term, written into the
          zero rows of mw);
      PE: h = W1am^T mw + W1c^T giT  (two K=128 matmuls accumulated in
          PSUM == relu-input of the reference MLP);
      ACT: h = relu(h + b1);  PE: y = w2^T h;  ACT: y = sigmoid(y + b2).
  - bf16 tables/matmuls (fp32 PSUM accumulation): measured end-to-end
    relative error ~1e-4 against the fp32 reference (tolerance 2e-2).
"""

import sys

sys.path.insert(0, "/opt/trn_rl_repo")

from contextlib import ExitStack

import numpy as np
import ml_dtypes

import concourse.bacc as bacc
import concourse.bass as bass
import concourse.tile as tile
from concourse import mybir
from concourse.bass_utils import run_bass_kernel_spmd

N_CORES = 8
P = 128
EMB = 32
B = 131_072
NUM_USERS = 1_000_000
NUM_ITEMS = 100_000
NUM_GROUPS = 500_000
ROWS_PER_CORE = B // N_CORES
NT = ROWS_PER_CORE // P  # 128 tiles per core
C = 4                    # tiles per compute sub-batch (n = 512)
N_QUEUES = 4

F32 = mybir.dt.float32
I32 = mybir.dt.int32
BF16 = mybir.dt.bfloat16
MULT = mybir.AluOpType.mult
ADD = mybir.AluOpType.add
BF = ml_dtypes.bfloat16


def indirect_dma_q(eng, out, in_, offset_ap, queue_num: int):
    """nc.gpsimd.indirect_dma_start clone with SWDGE queue selection."""
    assert in_.space == bass.MemorySpace.DRAM
    assert out.space == bass.MemorySpace.SBUF
    assert isinstance(in_.offset, int) and in_.offset == 0
    out_ap = eng.lower_ap_dma(out, for_indirect_dma=True)
    in_ap = eng.lower_ap_dma(in_, for_indirect_dma=True)
    assert len(in_ap) == 1 and len(out_ap) == 1
    offset_lowered = eng.lower_ap_dma(offset_ap)
    assert len(offset_lowered) == 1
    in_ap.append(offset_lowered[0])

    ap_shape = in_.shape
    coef = 1
    for i in range(1, len(ap_shape)):
        coef *= ap_shape[i]
    in_ap[0].dynamic_ap_info = mybir.DynamicAccessPatternInfo(
        c=0,
        actual_ap=out.ap,
        indirect_dim_max_index=ap_shape[0],
        offset_expr=[
            mybir.DynamicAccessPatternOffsetExpr(
                coef=coef,
                aff_expr=mybir.DynamicAccessPatternOffsetExprAffExpr(
                    kind="IndirectArgId", arg_id=1
                ),
            )
        ],
    )
    return eng.add_instruction(
        mybir.InstDMACopy(
            name=eng.bass.get_next_instruction_name(),
            queue=f"qPoolDynamic{queue_num or ''}",
            mode="Copy",
            ins=in_ap,
            outs=out_ap,
            oob_is_err=True,
            cce_op=mybir.AluOpType.bypass,
        )
    )


def build(nrows, n_queues=N_QUEUES):
    assert nrows % (P * C) == 0
    nt = nrows // P
    nsb = nt // C
    n = C * P  # columns per compute sub-batch

    nc = bacc.Bacc(
        "TRN2",
        target_bir_lowering=False,
        debug=False,
        enable_asserts=False,
        num_swdge_queues=max(n_queues, 1),
        # 64KB SWDGE descriptor-ring carveout: the default 16KB ring
        # throttles indirect-DMA issue (~13% per-DMA on a bare stream,
        # ~4% end-to-end); 96KB regresses (SBUF pressure).
        dynamic_dma_scratch_size=65536,
    )

    grp = nc.dram_tensor("grp_idx", [P, nt], I32, kind="ExternalInput")
    itm = nc.dram_tensor("item_idx", [P, nt], I32, kind="ExternalInput")
    trip_tab = nc.dram_tensor("trip_tab", [NUM_GROUPS, 96], BF16, kind="ExternalInput")
    item_tab = nc.dram_tensor("item_tab", [NUM_ITEMS, 32], BF16, kind="ExternalInput")
    # feature order everywhere on device: [item(0:32), mem0, mem1, mem2]
    we_d = nc.dram_tensor("we", [128, 128], BF16, kind="ExternalInput")
    bprime_d = nc.dram_tensor("bprime", [128, 1], F32, kind="ExternalInput")
    s_sel_d = nc.dram_tensor("s_sel", [128, 32], BF16, kind="ExternalInput")
    w1am_d = nc.dram_tensor("w1am", [128, 8], BF16, kind="ExternalInput")
    w1c_d = nc.dram_tensor("w1c", [128, 8], BF16, kind="ExternalInput")
    w2_d = nc.dram_tensor("w2", [8, 1], BF16, kind="ExternalInput")
    b1_d = nc.dram_tensor("b1", [8, 1], F32, kind="ExternalInput")
    b2_d = nc.dram_tensor("b2", [1, 1], F32, kind="ExternalInput")
    ident_d = nc.dram_tensor("ident", [P, P], BF16, kind="ExternalInput")
    y_out = nc.dram_tensor("y_out", [1, nrows], F32, kind="ExternalOutput")

    with tile.TileContext(nc) as tc, ExitStack() as ctx:
        singles = ctx.enter_context(tc.tile_pool(name="singles", bufs=1))
        gpool = ctx.enter_context(tc.tile_pool(name="gpool", bufs=3))
        spool = ctx.enter_context(tc.tile_pool(name="spool", bufs=2))
        pp_t = ctx.enter_context(tc.tile_pool(name="pp_t", bufs=2, space="PSUM"))
        pp_a = ctx.enter_context(tc.tile_pool(name="pp_a", bufs=2, space="PSUM"))
        pp_g = ctx.enter_context(tc.tile_pool(name="pp_g", bufs=1, space="PSUM"))
        pp_h = ctx.enter_context(tc.tile_pool(name="pp_h", bufs=1, space="PSUM"))
        pp_y = ctx.enter_context(tc.tile_pool(name="pp_y", bufs=1, space="PSUM"))

        def load(name, shape, dtype, dram):
            t = singles.tile(shape, dtype, tag=name)
            nc.sync.dma_start(out=t[:], in_=dram.ap())
            return t

        grp_all = load("grp_all", [P, nt], I32, grp)
        itm_all = load("itm_all", [P, nt], I32, itm)
        we = load("we", [128, 128], BF16, we_d)
        bprime = load("bprime", [128, 1], F32, bprime_d)
        s_sel = load("s_sel", [128, 32], BF16, s_sel_d)
        w1am = load("w1am", [128, 8], BF16, w1am_d)
        w1c = load("w1c", [128, 8], BF16, w1c_d)
        w2 = load("w2", [8, 1], BF16, w2_d)
        b1 = load("b1", [8, 1], F32, b1_d)
        b2 = load("b2", [1, 1], F32, b2_d)
        ident = load("ident", [P, P], BF16, ident_d)

        y_all = singles.tile([1, nrows], F32, tag="y_all")

        q = 0
        for sb in range(nsb):
            gi = gpool.tile([P, C, 128], BF16, tag="gi")
            for ti in range(C):
                t = sb * C + ti
                indirect_dma_q(
                    nc.gpsimd, gi[:, ti, 0:32], item_tab.ap(),
                    itm_all[:, t : t + 1], q % max(n_queues, 1),
                )
                q += 1
                indirect_dma_q(
                    nc.gpsimd, gi[:, ti, 32:128], trip_tab.ap(),
                    grp_all[:, t : t + 1], q % max(n_queues, 1),
                )
                q += 1

            gT_ps = pp_t.tile([P, n], BF16, tag="gT_ps")
            for ti in range(C):
                nc.tensor.transpose(
                    gT_ps[:, ti * P : (ti + 1) * P], gi[:, ti, :], ident[:]
                )
            gT = spool.tile([P, n], BF16, tag="gT")
            nc.vector.tensor_copy(out=gT[:], in_=gT_ps[:])

            # at = WE^T giT, rows 0:31 zero, rows 32+32k+d = logit k
            at_ps = pp_a.tile([128, n], F32, tag="at")
            nc.tensor.matmul(at_ps[:], lhsT=we[:], rhs=gT[:], start=True, stop=True)

            # mw = (at + b') * giT   (rows 32:128 = weighted members; 0:31 = 0)
            mw = spool.tile([128, n], BF16, tag="mw")
            nc.vector.scalar_tensor_tensor(
                out=mw[:], in0=at_ps[:], scalar=bprime[:], in1=gT[:],
                op0=ADD, op1=MULT,
            )

            # g = sum of the 3 member blocks of mw
            g_ps = pp_g.tile([32, n], F32, tag="g")
            nc.tensor.matmul(g_ps[:], lhsT=s_sel[:], rhs=mw[:], start=True, stop=True)

            # m1 = g * it written into mw rows 0:31 (zeros until now; the
            # s_sel matmul above already consumed mw, so WAR order holds)
            nc.vector.tensor_tensor(
                out=mw[0:32, :], in0=g_ps[:], in1=gT[0:32, :], op=MULT
            )

            # h = W1a^T m1 + W1b^T g + W1c^T it   (two K=128 matmuls)
            h_ps = pp_h.tile([8, n], F32, tag="h")
            nc.tensor.matmul(h_ps[:], lhsT=w1am[:], rhs=mw[:], start=True, stop=False)
            nc.tensor.matmul(h_ps[:], lhsT=w1c[:], rhs=gT[:], start=False, stop=True)

            h_sb = spool.tile([8, n], BF16, tag="h_sb")
            nc.scalar.activation(
                out=h_sb[:], in_=h_ps[:],
                func=mybir.ActivationFunctionType.Relu, bias=b1[:],
            )

            y_ps = pp_y.tile([1, n], F32, tag="y")
            nc.tensor.matmul(y_ps[:], lhsT=w2[:], rhs=h_sb[:], start=True, stop=True)

            nc.scalar.activation(
                out=y_all[0:1, sb * n : (sb + 1) * n], in_=y_ps[:],
                func=mybir.ActivationFunctionType.Sigmoid, bias=b2[:],
            )

        nc.sync.dma_start(out=y_out.ap(), in_=y_all[:])

    nc.compile()
    return nc


def prep_host_inputs(inputs):
    grp = np.asarray(inputs["group_inputs"]).astype(np.int32).reshape(-1)
    itm = np.asarray(inputs["item_inputs"]).astype(np.int32).reshape(-1)
    members = np.asarray(inputs["members"]).astype(np.int64)
    user_emb = np.asarray(inputs["user_emb"], np.float32)
    trip_tab = np.ascontiguousarray(
        user_emb[members].reshape(NUM_GROUPS, 3 * EMB)
    ).astype(BF)
    item_tab = np.ascontiguousarray(
        np.concatenate(
            [
                np.asarray(inputs["item_emb"], np.float32),
                np.asarray(inputs["genres"], np.float32),
            ],
            axis=1,
        )
    ).astype(BF)

    attn_W = np.asarray(inputs["attn_W"], np.float32)  # [128, 3]
    attn_b = np.asarray(inputs["attn_b"], np.float32)  # [3]
    W1 = np.asarray(inputs["pred_W1"], np.float32)     # [96, 8]
    b1 = np.asarray(inputs["pred_b1"], np.float32)     # [8]
    w2 = np.asarray(inputs["pred_W2"], np.float32)     # [8, 1]
    b2 = np.asarray(inputs["pred_b2"], np.float32)     # [1]

    # device feature order f: [item(0:32), mem0, mem1, mem2]; the reference
    # gi order is [mem0, mem1, mem2, item], so permute attn_W rows.
    attn_W_perm = np.concatenate([attn_W[3 * EMB :], attn_W[: 3 * EMB]], axis=0)
    rep = np.repeat(np.arange(3), EMB)  # [96]
    z32 = np.zeros((EMB, 1), np.float32)
    we = np.zeros((128, 128), np.float32)
    we[:, EMB:] = attn_W_perm[:, rep]            # cols 32+32k+d -> logit k
    bprime = np.concatenate([z32, attn_b[rep].reshape(96, 1)])
    s_sel = np.concatenate(
        [np.zeros((EMB, EMB), np.float32), np.tile(np.eye(EMB, dtype=np.float32), (3, 1))]
    )
    w1am = np.concatenate([W1[0:EMB], np.tile(W1[EMB : 2 * EMB], (3, 1))])
    w1c = np.concatenate([W1[2 * EMB : 3 * EMB], np.zeros((96, 8), np.float32)])
    w = {
        "we": np.ascontiguousarray(we).astype(BF),
        "bprime": np.ascontiguousarray(bprime).astype(np.float32),
        "s_sel": np.ascontiguousarray(s_sel).astype(BF),
        "w1am": np.ascontiguousarray(w1am).astype(BF),
        "w1c": np.ascontiguousarray(w1c).astype(BF),
        "w2": np.ascontiguousarray(w2).astype(BF),
        "b1": np.ascontiguousarray(b1.reshape(8, 1)).astype(np.float32),
        "b2": np.ascontiguousarray(b2.reshape(1, 1)).astype(np.float32),
        "ident": np.eye(P, dtype=np.float32).astype(BF),
    }
    return grp, itm, trip_tab, item_tab, w


def make_in_maps(grp, itm, trip_tab, item_tab, w, nrows, n_cores):
    nt = nrows // P
    in_maps = []
    for c in range(n_cores):
        sl = slice(c * nrows, (c + 1) * nrows)
        in_maps.append(
            {
                # tile t holds rows [t*128, (t+1)*128); partition p = row t*128+p
                "grp_idx": np.ascontiguousarray(grp[sl].reshape(nt, P).T),
                "item_idx": np.ascontiguousarray(itm[sl].reshape(nt, P).T),
                "trip_tab": trip_tab,
                "item_tab": item_tab,
                **w,
            }
        )
    return in_maps


_NC_CACHE = {}


def kernel(**inputs) -> np.ndarray:
    grp, itm, trip_tab, item_tab, w = prep_host_inputs(inputs)
    if ROWS_PER_CORE not in _NC_CACHE:
        _NC_CACHE[ROWS_PER_CORE] = build(ROWS_PER_CORE)
    nc = _NC_CACHE[ROWS_PER_CORE]
    in_maps = make_in_maps(grp, itm, trip_tab, item_tab, w, ROWS_PER_CORE, N_CORES)
    res = run_bass_kernel_spmd(nc, in_maps, core_ids=list(range(N_CORES)))
    outs = [res.results[c]["y_out"].reshape(ROWS_PER_CORE) for c in range(N_CORES)]
    return np.concatenate(outs).reshape(B, 1).astype(np.float32)
